# revision 10
# baseline (speedup 1.0000x reference)
"""BCQ linear kernel for 8 TRN2 NeuronCores.

y = x @ dequant(qweight, alpha, beta)
  x: (4, 2048, 4096) f32, qweight: (128, 4, 4096) i32 bit-planes,
  alpha: (32, 4, 4096) f32, beta: (32, 4096) f32 -> y: (4, 2048, 4096) f32

Strategy: tensor-parallel over out_features (512 per core). Each core:
  - dequantizes its w slice (4096 x 512) on-chip into bf16 SBUF
      w[k, o] = sum_b alpha[g,b,o] * sign(bit_b(k,o)) + beta[g,o]
             = sum_b 2*alpha[g,b,o] * bit_b(k,o) + (beta[g,o] - sum_b alpha[g,b,o])
    bit extraction: host pre-shifts each packed word so that partition p's
    bit sits in the int16 sign position; on-chip it is a single
    (qs < 0) * (2*alpha) fused scalar_tensor_tensor op per k-tile.
  - streams x^T (bf16, host-prepared) and matmuls: out[m,o] tiles with
    lhsT = x^T[k,m] (stationary), rhs = w[k,o] (moving), f32 PSUM accum.
Host gathers the 8 out-feature slices.
"""
import sys

if "/opt/trn_rl_repo" not in sys.path:
    sys.path.insert(0, "/opt/trn_rl_repo")

import numpy as np
from ml_dtypes import bfloat16

import concourse.bacc as bacc
import concourse.bass as bass
import concourse.tile as tile
from concourse import mybir
from concourse.bass_utils import run_bass_kernel_spmd

IN_F = 4096
OUT_F = 4096
GROUP_SIZE = 128
WB = 4
BATCH = 4
SEQ = 2048
M_FULL = BATCH * SEQ          # 8192
N_CORES = 8
O_SH = OUT_F // N_CORES       # 512
P = 128

F32 = mybir.dt.float32
BF16 = mybir.dt.bfloat16
I32 = mybir.dt.int32
I16 = mybir.dt.int16
Alu = mybir.AluOpType


def build(M=M_FULL, K=IN_F, O=O_SH, debug=False):
    """Build the per-core Bass graph (SPMD: same graph, per-core inputs)."""
    assert M % 512 == 0 and K % P == 0
    KT = K // P                # k tiles (= quant groups, GROUP_SIZE == P)
    MC = M // 512              # m chunks of 512 rows (4 m-tiles each)
    P1C = min(2, MC)           # chunks processed k-outer during dequant

    nc = bacc.Bacc(None, target_bir_lowering=False, debug=debug)

    MC_ = M // 512
    GP = KT // 2               # k-tile pairs (dequant batch unit)
    xt_d = nc.dram_tensor("xt", (MC_, P, KT, 512), BF16, kind="ExternalInput")
    qs_d = nc.dram_tensor("qs", (GP, P, 2, WB, O), I32, kind="ExternalInput")
    alc_d = nc.dram_tensor("alc", (GP, 2, WB + 1, O), BF16, kind="ExternalInput")
    out_d = nc.dram_tensor("out", (M, O), F32, kind="ExternalOutput")

    def load_x_chunk(pool, mc):
        """One m-chunk of x^T (host-tiled: contiguous 16KB per partition)."""
        xt_sb = pool.tile([P, KT, 512], BF16, name=f"xt_sb{mc}", tag="xt")
        nc.sync.dma_start(out=xt_sb[:], in_=xt_d[mc, :, :, :])
        return xt_sb

    with tile.TileContext(nc) as tc:
        with (
            tc.tile_pool(name="wpool", bufs=1) as wpool,
            tc.tile_pool(name="dqi", bufs=2) as dqi,
            tc.tile_pool(name="dq", bufs=2) as dq,
            tc.tile_pool(name="xs", bufs=2) as xs,
            tc.tile_pool(name="ys", bufs=4) as ys,
            tc.tile_pool(name="ps", bufs=8, space="PSUM") as ps,
        ):
            w_pairs = [
                wpool.tile([P, 2, O], BF16, name=f"w{gp}", tag=f"w{gp}")
                for gp in range(GP)
            ]

            def w_rhs(g):
                return w_pairs[g // 2][:, g % 2, :]

            # prefetch the first dequant inputs ahead of the big x DMAs
            PREF = min(2, GP)
            qts, abs_ = {}, {}

            def load_dq(gp):
                qt = dqi.tile([P, 2, WB, O], I32, name=f"qt{gp}", tag="qt")
                nc.sync.dma_start(out=qt[:], in_=qs_d[gp])
                ab = dqi.tile([P, 2, WB + 1, O], BF16, name=f"ab{gp}", tag="ab")
                alc_src = bass.AP(
                    tensor=alc_d[:, :, :, :].tensor,
                    offset=gp * 2 * (WB + 1) * O,
                    ap=[[0, P], [(WB + 1) * O, 2], [O, WB + 1], [1, O]],
                )
                nc.gpsimd.dma_start(out=ab[:], in_=alc_src)
                qts[gp], abs_[gp] = qt, ab

            for gp in range(PREF):
                load_dq(gp)

            x_chunks = {mc: load_x_chunk(xs, mc) for mc in range(P1C)}

            psum_p1 = [
                ps.tile([P, O], F32, name=f"ps{i}", tag="ps")
                for i in range(4 * P1C)
            ]

            # ---- phase 1: dequant k-tile pairs; matmul first P1C chunks ----
            for gp in range(GP):
                qt, ab = qts[gp], abs_[gp]
                # t[b] = (qs[b] < 0) * 2alpha[b]   (sign bit = the quant bit)
                t = dq.tile([P, 2, WB, O], BF16, tag="t")
                nc.vector.scalar_tensor_tensor(
                    out=t[:], in0=qt[:], scalar=0.0, in1=ab[:, :, 0:WB, :],
                    op0=Alu.is_lt, op1=Alu.mult,
                )
                # bit-plane sum + C on GpSimd (keeps DVE free for the stt)
                s = dq.tile([P, 2, 2, O], BF16, tag="s")
                nc.gpsimd.tensor_tensor(
                    s[:], t[:, :, 0:2, :], t[:, :, 2:4, :], Alu.add
                )
                s2 = dq.tile([P, 2, O], BF16, tag="s2")
                nc.gpsimd.tensor_tensor(
                    s2[:], s[:, :, 0, :], s[:, :, 1, :], Alu.add
                )
                nc.gpsimd.tensor_tensor(
                    w_pairs[gp][:], s2[:], ab[:, :, WB, :], Alu.add
                )
                if gp + PREF < GP:
                    load_dq(gp + PREF)

                # matmul this k-tile pair into the first P1C chunks' psums
                for j in range(2):
                    g = gp * 2 + j
                    for mc in range(P1C):
                        for mt in range(4):
                            nc.tensor.matmul(
                                psum_p1[mc * 4 + mt][:],
                                x_chunks[mc][:, g, mt * 128:(mt + 1) * 128],
                                w_rhs(g),
                                start=(g == 0),
                                stop=(g == KT - 1),
                            )

            for mc in range(P1C):
                for mt in range(4):
                    y_sb = ys.tile([P, O], F32, tag="y")
                    nc.scalar.copy(y_sb[:], psum_p1[mc * 4 + mt][:])
                    row = (mc * 4 + mt) * 128
                    nc.sync.dma_start(out=out_d[row:row + 128, :], in_=y_sb[:])

            # ---- phase 2: remaining m chunks at full speed ----
            for mc in range(P1C, MC):
                xt_sb = load_x_chunk(xs, mc)
                for mt in range(4):
                    psum = ps.tile([P, O], F32, tag="ps")
                    for g in range(KT):
                        nc.tensor.matmul(
                            psum[:],
                            xt_sb[:, g, mt * 128:(mt + 1) * 128],
                            w_rhs(g),
                            start=(g == 0),
                            stop=(g == KT - 1),
                        )
                    y_sb = ys.tile([P, O], F32, tag="y")
                    nc.scalar.copy(y_sb[:], psum[:])
                    row = (mc * 4 + mt) * 128
                    nc.sync.dma_start(out=out_d[row:row + 128, :], in_=y_sb[:])

    return nc


def host_prep(x, qweight, alpha, beta, M=M_FULL, K=IN_F):
    """Full inputs -> per-core in_maps (shard over out_features)."""
    KT = K // P
    MC = M // 512
    KT_ = K // P
    x3 = x.reshape(M, K).astype(bfloat16)
    # (MC, P, KT, 512): per-partition-contiguous chunk tiles for fast DMA
    x2 = np.ascontiguousarray(
        x3.reshape(MC, 512, KT_, P).transpose(0, 3, 2, 1)
    )

    k = np.arange(K)
    widx = (k // 32).astype(np.int64)
    shl = (31 - (k % 32)).astype(np.int32)

    o_sh = qweight.shape[-1] // N_CORES
    in_maps = []
    for c in range(N_CORES):
        sl = slice(c * o_sh, (c + 1) * o_sh)
        qw_s = qweight[:, :, sl]                       # (K/32, WB, o_sh) i32
        qs = (qw_s[widx] << shl[:, None, None]).astype(np.int32)
        qs = qs.reshape(KT // 2, 2, P, WB, o_sh).transpose(0, 2, 1, 3, 4)
        qs = np.ascontiguousarray(qs)                  # (GP, P, 2, WB, o)
        al_s = alpha[:, :, sl].astype(np.float32)
        alc = np.empty((KT, WB + 1, o_sh), dtype=bfloat16)
        alc[:, :WB, :] = (2.0 * al_s).astype(bfloat16)
        alc[:, WB, :] = (
            beta[:, sl].astype(np.float32) - al_s.sum(axis=1)
        ).astype(bfloat16)
        alc = alc.reshape(KT // 2, 2, WB + 1, o_sh)    # (GP, 2, 5, o)
        in_maps.append({"xt": x2, "qs": qs, "alc": np.ascontiguousarray(alc)})
    return in_maps


_NC_CACHE = {}


def _get_nc():
    if "nc" not in _NC_CACHE:
        nc = build()
        nc.compile()
        _NC_CACHE["nc"] = nc
    return _NC_CACHE["nc"]


def run(x, qweight, alpha, beta, trace=False, **kwargs):
    nc = _get_nc()
    in_maps = host_prep(x, qweight, alpha, beta)
    res = run_bass_kernel_spmd(
        nc, in_maps, core_ids=list(range(N_CORES)), trace=trace, **kwargs
    )
    y = np.concatenate(
        [np.asarray(res.results[c]["out"]) for c in range(N_CORES)], axis=1
    )
    y = np.ascontiguousarray(y.astype(np.float32)).reshape(BATCH, SEQ, OUT_F)
    return y, res


def kernel(x, qweight, alpha, beta):
    y, _ = run(
        np.asarray(x), np.asarray(qweight), np.asarray(alpha), np.asarray(beta)
    )
    return y


# revision 11
# speedup vs baseline: 1.0525x; 1.0525x over previous
"""BCQ linear kernel for 8 TRN2 NeuronCores.

y = x @ dequant(qweight, alpha, beta)
  x: (4, 2048, 4096) f32, qweight: (128, 4, 4096) i32 bit-planes,
  alpha: (32, 4, 4096) f32, beta: (32, 4096) f32 -> y: (4, 2048, 4096) f32

Strategy: tensor-parallel over out_features (512 per core). Each core:
  - dequantizes its w slice (4096 x 512) on-chip into bf16 SBUF
      w[k, o] = sum_b alpha[g,b,o] * sign(bit_b(k,o)) + beta[g,o]
             = sum_b 2*alpha[g,b,o] * bit_b(k,o) + (beta[g,o] - sum_b alpha[g,b,o])
    bit extraction: host pre-shifts each packed word so that partition p's
    bit sits in the int32 sign position; on-chip it is a single
    (qs < 0) * (2*alpha) fused scalar_tensor_tensor op per k-tile.
  - matmuls in NR=4 k-rounds of 8 k-tiles each; every round sweeps all of
    M with 8-matmul PSUM groups and accumulates partial sums into a bf16
    SBUF accumulator (middle rounds via SBUF->SBUF accumulate-DMA, final
    round adds on VectorE and writes f32).  Closing PSUM groups every 8
    matmuls lets the PE trail the dequant frontier instead of stalling all
    8 PSUM banks on it, so dequant is almost fully hidden.
  - x^T is host-prepared bf16, tiled (round, chunk, partition-contiguous)
    for full-rate DMA.
Host gathers the 8 out-feature slices.
"""
import sys

if "/opt/trn_rl_repo" not in sys.path:
    sys.path.insert(0, "/opt/trn_rl_repo")

import numpy as np
from ml_dtypes import bfloat16

import concourse.bacc as bacc
import concourse.bass as bass
import concourse.tile as tile
from concourse import mybir
from concourse.bass_utils import run_bass_kernel_spmd

IN_F = 4096
OUT_F = 4096
GROUP_SIZE = 128
WB = 4
BATCH = 4
SEQ = 2048
M_FULL = BATCH * SEQ          # 8192
N_CORES = 8
O_SH = OUT_F // N_CORES       # 512
P = 128
NR = 4                        # k rounds
MCH = 256                     # m chunk rows

F32 = mybir.dt.float32
BF16 = mybir.dt.bfloat16
I32 = mybir.dt.int32
Alu = mybir.AluOpType


def build(M=M_FULL, K=IN_F, O=O_SH, debug=False):
    """Build the per-core Bass graph (SPMD: same graph, per-core inputs)."""
    assert M % MCH == 0 and K % (P * NR) == 0
    KT = K // P                # k tiles (= quant groups, GROUP_SIZE == P)
    KR = KT // NR              # k tiles per round
    MC = M // MCH              # m chunks
    MTPC = MCH // 128          # m tiles per chunk
    MT = M // 128              # m tiles

    nc = bacc.Bacc(None, target_bir_lowering=False, debug=debug)

    xt_d = nc.dram_tensor("xt", (NR, MC, P, KR, MCH), BF16, kind="ExternalInput")
    qs_d = nc.dram_tensor("qs", (KT, P, WB, O), I32, kind="ExternalInput")
    alc_d = nc.dram_tensor("alc", (KT, WB + 1, O), BF16, kind="ExternalInput")
    out_d = nc.dram_tensor("out", (M, O), F32, kind="ExternalOutput")

    with tile.TileContext(nc) as tc:
        with (
            tc.tile_pool(name="wpool", bufs=1) as wpool,
            tc.tile_pool(name="ypool", bufs=1) as ypool,
            tc.tile_pool(name="dqi", bufs=4) as dqi,
            tc.tile_pool(name="dq", bufs=3) as dq,
            tc.tile_pool(name="xs", bufs=3) as xs,
            tc.tile_pool(name="ys", bufs=6) as ys,
            tc.tile_pool(name="ps", bufs=8, space="PSUM") as ps,
        ):
            w_tiles = [
                wpool.tile([P, O], BF16, name=f"w{g}", tag=f"w{g}")
                for g in range(KT)
            ]
            yacc = [
                ypool.tile([P, O], BF16, name=f"ya{t}", tag=f"ya{t}")
                for t in range(MT)
            ]

            # streamed dequant-input DMAs (prefetched ahead of consumption)
            PREF = min(4, KT)
            qts, abs_ = {}, {}

            def load_dq(g):
                qt = dqi.tile([P, WB, O], I32, name=f"qt{g}", tag="qt")
                nc.sync.dma_start(out=qt[:], in_=qs_d[g, :, :, :])
                ab = dqi.tile([P, WB + 1, O], BF16, name=f"ab{g}", tag="ab")
                alc_src = bass.AP(
                    tensor=alc_d[:, :, :].tensor,
                    offset=g * (WB + 1) * O,
                    ap=[[0, P], [O, WB + 1], [1, O]],
                )
                nc.gpsimd.dma_start(out=ab[:], in_=alc_src)
                qts[g], abs_[g] = qt, ab

            def dequant(g):
                qt, ab = qts[g], abs_[g]
                # t[b] = (qs[b] < 0) * 2alpha[b]  (sign bit = the quant bit)
                t = dq.tile([P, WB, O], BF16, tag="t")
                nc.vector.scalar_tensor_tensor(
                    out=t[:], in0=qt[:], scalar=0.0, in1=ab[:, 0:WB, :],
                    op0=Alu.is_lt, op1=Alu.mult,
                )
                s = dq.tile([P, 2, O], BF16, tag="s")
                nc.vector.tensor_tensor(s[:], t[:, 0:2, :], t[:, 2:4, :], Alu.add)
                s2 = dq.tile([P, O], BF16, tag="s2")
                nc.vector.tensor_tensor(s2[:], s[:, 0, :], s[:, 1, :], Alu.add)
                nc.vector.tensor_tensor(
                    w_tiles[g][:], s2[:], ab[:, WB, :], Alu.add
                )
                if g + PREF < KT:
                    load_dq(g + PREF)

            for g in range(PREF):
                load_dq(g)

            gdq = 0  # next k-tile to dequantize
            for r in range(NR):
                while gdq < (r + 1) * KR:
                    dequant(gdq)
                    gdq += 1
                for c in range(MC):
                    x_t = xs.tile([P, KR, MCH], BF16, name=f"x{r}_{c}", tag="x")
                    nc.sync.dma_start(out=x_t[:], in_=xt_d[r, c])
                    for mt in range(MTPC):
                        mti = c * MTPC + mt
                        psum = ps.tile([P, O], F32, tag="ps")
                        for j in range(KR):
                            nc.tensor.matmul(
                                psum[:],
                                x_t[:, j, mt * 128:(mt + 1) * 128],
                                w_tiles[r * KR + j][:],
                                start=(j == 0),
                                stop=(j == KR - 1),
                            )
                        if r == 0:
                            nc.scalar.copy(yacc[mti][:], psum[:])
                        elif r < NR - 1:
                            dtile = ys.tile([P, O], BF16, tag="yd")
                            nc.scalar.copy(dtile[:], psum[:])
                            nc.gpsimd.dma_start(
                                out=yacc[mti][:], in_=dtile[:],
                                accum_op=Alu.add,
                            )
                        else:
                            y_sb = ys.tile([P, O], F32, tag="y")
                            nc.vector.tensor_tensor(
                                y_sb[:], psum[:], yacc[mti][:], Alu.add
                            )
                            nc.sync.dma_start(
                                out=out_d[mti * 128:(mti + 1) * 128, :],
                                in_=y_sb[:],
                            )

    return nc


def host_prep(x, qweight, alpha, beta, M=M_FULL, K=IN_F):
    """Full inputs -> per-core in_maps (shard over out_features)."""
    KT = K // P
    KR = KT // NR
    MC = M // MCH
    x3 = x.reshape(M, K).astype(bfloat16)
    # (NR, MC, P, KR, MCH): per-(round,chunk) partition-contiguous x^T tiles
    x2 = np.ascontiguousarray(
        x3.reshape(MC, MCH, NR, KR, P).transpose(2, 0, 4, 3, 1)
    )

    k = np.arange(K)
    widx = (k // 32).astype(np.int64)
    shl = (31 - (k % 32)).astype(np.int32)

    o_sh = qweight.shape[-1] // N_CORES
    in_maps = []
    for c in range(N_CORES):
        sl = slice(c * o_sh, (c + 1) * o_sh)
        qw_s = qweight[:, :, sl]                       # (K/32, WB, o_sh) i32
        qs = (qw_s[widx] << shl[:, None, None]).astype(np.int32)
        qs = np.ascontiguousarray(qs.reshape(KT, P, WB, o_sh))
        al_s = alpha[:, :, sl].astype(np.float32)
        alc = np.empty((KT, WB + 1, o_sh), dtype=bfloat16)
        alc[:, :WB, :] = (2.0 * al_s).astype(bfloat16)
        alc[:, WB, :] = (
            beta[:, sl].astype(np.float32) - al_s.sum(axis=1)
        ).astype(bfloat16)
        in_maps.append({"xt": x2, "qs": qs, "alc": np.ascontiguousarray(alc)})
    return in_maps


_NC_CACHE = {}


def _get_nc():
    if "nc" not in _NC_CACHE:
        nc = build()
        nc.compile()
        _NC_CACHE["nc"] = nc
    return _NC_CACHE["nc"]


def run(x, qweight, alpha, beta, trace=False, **kwargs):
    nc = _get_nc()
    in_maps = host_prep(x, qweight, alpha, beta)
    res = run_bass_kernel_spmd(
        nc, in_maps, core_ids=list(range(N_CORES)), trace=trace, **kwargs
    )
    y = np.concatenate(
        [np.asarray(res.results[c]["out"]) for c in range(N_CORES)], axis=1
    )
    y = np.ascontiguousarray(y.astype(np.float32)).reshape(BATCH, SEQ, OUT_F)
    return y, res


def kernel(x, qweight, alpha, beta):
    y, _ = run(
        np.asarray(x), np.asarray(qweight), np.asarray(alpha), np.asarray(beta)
    )
    return y
